# revision 15
# baseline (speedup 1.0000x reference)
"""Multi-head self-attention Trainium2 kernel (B=4, S=4096, D=256, H=4).

Sharding: 8 cores = batch (4) x query-half (2). Each core receives the
full sequence of its batch, rotated so that its 2048 query tokens are
tokens 0..2047 (softmax + PV are invariant to key order, so rotating the
key sequence is legal and keeps the SPMD program identical across cores).

Per-core layout choices:
  - X^T, Q^T, K^T in [feature, token] layout (feature on partitions) so
    the attention matmuls contract over the head dim.
  - scores^T in [key, query] layout: stationary = K^T chunk [64, 128],
    moving = Q^T [64, 512]; heads of a pair sit at partitions 0:64 /
    64:128 so their matmuls run concurrently on disjoint PE row groups.
  - exp() on the scalar engine straight out of PSUM (scale=1/8 folded in).
  - PV: stationary = V-chunk [128 keys, cols] in natural token-major
    layout with an extra ones-column, so the same accumulating matmul
    also produces the softmax denominator l.  Even heads use cols
    [V|1] -> attn at psum partitions 0:64, l at 64; odd heads use
    [0...0|1|V] (M=128) -> l at partition 63, attn at 64:128, which
    stacks the head pair into the out-projection stationary layout
    without cross-partition copies.
  - 1/l replication across the 128 stacked partitions via a tiny K=2
    matmul with an indicator stationary.
"""

import numpy as np

import concourse.bass as bass
import concourse.bacc as bacc
import concourse.tile as tile
from concourse import mybir
from concourse.masks import make_identity

B, S, D, H = 4, 4096, 256, 4
Dh = D // H  # 64
NCORES = 8
SQ = S // 2  # queries per core (2048)

F32 = mybir.dt.float32

# V-augmented column layout: per pair of heads (even, odd):
#   even head h: 65 cols  [V_h (64) | ones]           -> attn @ psum 0:64,  l @ 64
#   odd  head h: 128 cols [0*32 | ones | 0*31 | V_h]  -> l @ 32, attn @ 64:128
# (l columns sit at 32-aligned psum partitions so the K=1 ones-stationary
#  replication matmuls below have legal tile_positions)
PAIR_COLS = 65 + 128  # 193
VAUG_COLS = 2 * PAIR_COLS  # 386


def _even_off(pair):
    return pair * PAIR_COLS


def _odd_off(pair):
    return pair * PAIR_COLS + 65


# softmax-exp tiles handled by the DVE (2-pass custom op) instead of ACT,
# to split the exp work across both engines; rest go to ACT
DVE_EXP_KCS = frozenset({3, 6, 9, 13, 16, 19, 23, 26, 29})

_EXP_OPS = None


def _register_exp_ops():
    """Register two custom DVE ops computing exp(s/8) in two passes:
    pass1: p = poly3(s/1024) ~= exp(s/1024); pass2: p^128 via 7 squarings.
    Max rel err ~7e-5 over s in [-60, 60]. Registered at runtime (appended
    to dve_ops.OPS) so kernel.py stays self-contained."""
    global _EXP_OPS
    if _EXP_OPS is not None:
        return _EXP_OPS
    import concourse.dve_ops as dve_ops
    from concourse.dve_spec import Spec, Src0, C0, C1, C2, One, sq, lower, _has_src1
    from concourse.dve_uop import DveOpSpec

    def mk(name, body, ref):
        if name in dve_ops._SUB_OPCODE_FOR_NAME:
            return next(o for o in dve_ops.OPS if o.name == name)
        row = dve_ops._CUSTOM_DVE_ROW_BASE + len(dve_ops.OPS)
        assert row < 0x20
        spec = Spec(body=body, reference=ref)
        shas = {}
        for ver in ("v3", "v4"):
            s = DveOpSpec(name=name, opcode=row, uops=lower(spec, ver=ver),
                          rd1_en=_has_src1(spec))
            shas[ver] = s.sha(ver)
        op = dve_ops.DveOp(name, spec, subdim=False, uops_sha=shas)
        dve_ops.OPS.append(op)
        dve_ops.CUSTOM_DVE_SPECS[name] = spec
        dve_ops._SUB_OPCODE_FOR_NAME[name] = row
        return op

    t = Src0 * C0
    body1 = ((t * C1 + C2) * t + One) * t + One

    def ref1(in0, in1, s0, s1, imm2):
        tt = (in0 * s0).astype(np.float32)
        return (((tt * s1 + imm2) * tt + 1.0) * tt + 1.0).astype(np.float32)

    x = Src0
    for _ in range(7):
        x = sq(x)

    def ref2(in0, in1, s0, s1, imm2):
        y = in0
        for _ in range(7):
            y = (y * y).astype(np.float32)
        return y

    op1 = mk("ANT_EXP_P3", body1, ref1)
    op2 = mk("ANT_SQ7", x, ref2)
    _EXP_OPS = (op1, op2)
    return _EXP_OPS


def build_program():
    nc = bacc.Bacc("TRN2", target_bir_lowering=False, debug=False)

    x = nc.declare_dram_parameter("x", [S, D], F32, isOutput=False)
    wq = nc.declare_dram_parameter("wq", [D, D], F32, isOutput=False)
    bq = nc.declare_dram_parameter("bq", [D], F32, isOutput=False)
    wk = nc.declare_dram_parameter("wk", [D, D], F32, isOutput=False)
    bk = nc.declare_dram_parameter("bk", [D], F32, isOutput=False)
    wv = nc.declare_dram_parameter("wv", [D, D], F32, isOutput=False)
    bv = nc.declare_dram_parameter("bv", [D], F32, isOutput=False)
    wo = nc.declare_dram_parameter("wo", [D, D], F32, isOutput=False)
    bo = nc.declare_dram_parameter("bo", [D], F32, isOutput=False)
    out = nc.declare_dram_parameter("out", [SQ, D], F32, isOutput=True)

    with tile.TileContext(nc) as tc:
        _emit(nc, tc, x, wq, bq, wk, bk, wv, bv, wo, bo, out)
    nc.finalize()  # Bacc.finalize runs the compile passes (reg alloc,
    # matmul-wait splitting) that walrus codegen requires
    return nc


def _emit(nc, tc, x, wq, bq, wk, bk, wv, bv, wo, bo, out):
    from contextlib import ExitStack

    ctx = ExitStack()
    with ctx:
        singles = ctx.enter_context(tc.tile_pool(name="singles", bufs=1))

        # ---- constants -------------------------------------------------
        ident = singles.tile([128, 128], F32)
        make_identity(nc, ident)

        # weight tiles, natural [d_chunk(128 part), 256] layout
        wq_sb = singles.tile([128, 2, D], F32)
        wk_sb = singles.tile([128, 2, D], F32)
        wo_sb = singles.tile([128, 2, D], F32)
        wv_sb = singles.tile([128, 2, D], F32)
        for dc in range(2):
            nc.sync.dma_start(out=wq_sb[:, dc, :], in_=wq[dc * 128:(dc + 1) * 128, :])
            nc.sync.dma_start(out=wk_sb[:, dc, :], in_=wk[dc * 128:(dc + 1) * 128, :])
            nc.sync.dma_start(out=wo_sb[:, dc, :], in_=wo[dc * 128:(dc + 1) * 128, :])
            nc.sync.dma_start(out=wv_sb[:, dc, :], in_=wv[dc * 128:(dc + 1) * 128, :])

        # augmented Wv' [d_chunk, VAUG_COLS]
        wva_sb = singles.tile([128, 2, VAUG_COLS], F32)
        nc.vector.memset(wva_sb, 0.0)
        for dc in range(2):
            for h in range(H):
                pair, par = h // 2, h % 2
                off = (_even_off(pair) if par == 0 else _odd_off(pair) + 64)
                nc.vector.tensor_copy(
                    out=wva_sb[:, dc, off:off + Dh],
                    in_=wv_sb[:, dc, h * Dh:(h + 1) * Dh],
                )

        # bias columns [128, 1] per feature chunk (partition-major load)
        bq_col = singles.tile([128, 2], F32)
        bk_col = singles.tile([128, 2], F32)
        for fc in range(2):
            nc.gpsimd.dma_start(out=bq_col[:, fc:fc + 1], in_=bq[fc * 128:(fc + 1) * 128])
            nc.gpsimd.dma_start(out=bk_col[:, fc:fc + 1], in_=bk[fc * 128:(fc + 1) * 128])

        # broadcast rows (same value in every partition)
        def bcast_row(src, n, tag):
            t = singles.tile([128, n], F32, tag=tag)
            src_b = bass.AP(tensor=src.tensor, offset=src.offset,
                            ap=[[0, 128]] + list(src.ap))
            nc.gpsimd.dma_start(out=t, in_=src_b)
            return t

        bo_bc = bcast_row(bo[:], D, "bo_bc")
        bv_bc = bcast_row(bv[:], D, "bv_bc")

        # augmented bias row for V': V cols get bv, "ones" cols get 1.0
        bva_bc = singles.tile([128, VAUG_COLS], F32)
        nc.vector.memset(bva_bc, 0.0)
        for h in range(H):
            pair, par = h // 2, h % 2
            off = (_even_off(pair) if par == 0 else _odd_off(pair) + 64)
            one_off = (_even_off(pair) + Dh if par == 0 else _odd_off(pair) + 32)
            nc.vector.tensor_copy(out=bva_bc[:, off:off + Dh],
                                  in_=bv_bc[:, h * Dh:(h + 1) * Dh])
            nc.vector.memset(bva_bc[:, one_off:one_off + 1], 1.0)

        # all-ones tile: K=1 stationary rows for the 1/l replication matmuls
        ones128 = singles.tile([128, 128], F32)
        nc.vector.memset(ones128, 1.0)

        # ---- persistent activations -----------------------------------
        xT = singles.tile([128, 2, S], F32)    # X^T  [d, t]
        kT = singles.tile([128, 2, S], F32)    # K^T  [f, t]
        qT = singles.tile([128, 2, SQ], F32)   # Q^T  [f, t<SQ]
        v_sb = singles.tile([128, S // 128, VAUG_COLS], F32)  # V' natural

        # ---- phase B: X^T via PE transpose ----------------------------
        with tc.tile_pool(name="xload", bufs=3) as xload, \
             tc.tile_pool(name="tpsum", bufs=4, space="PSUM") as tpsum:
            for t in range(S // 128):
                xt = xload.tile([128, D], F32)
                nc.sync.dma_start(out=xt, in_=x[t * 128:(t + 1) * 128, :])
                for dc in range(2):
                    pt = tpsum.tile([128, 128], F32)
                    nc.tensor.transpose(pt, xt[:, dc * 128:(dc + 1) * 128], ident)
                    nc.vector.tensor_copy(
                        out=xT[:, dc, t * 128:(t + 1) * 128], in_=pt)

        # ---- phase C: projections -------------------------------------
        with tc.tile_pool(name="ppsum", bufs=4, space="PSUM") as ppsum:
            # K^T (full S) and Q^T (first SQ tokens)
            for fc in range(2):
                for tt in range(S // 512):
                    ps = ppsum.tile([128, 512], F32, tag="proj")
                    for dc in range(2):
                        nc.tensor.matmul(
                            ps,
                            lhsT=wk_sb[:, dc, fc * 128:(fc + 1) * 128],
                            rhs=xT[:, dc, tt * 512:(tt + 1) * 512],
                            start=(dc == 0), stop=(dc == 1))
                    nc.vector.tensor_scalar_add(
                        out=kT[:, fc, tt * 512:(tt + 1) * 512],
                        in0=ps, scalar1=bk_col[:, fc:fc + 1])
                for tt in range(SQ // 512):
                    ps = ppsum.tile([128, 512], F32, tag="proj")
                    for dc in range(2):
                        nc.tensor.matmul(
                            ps,
                            lhsT=wq_sb[:, dc, fc * 128:(fc + 1) * 128],
                            rhs=xT[:, dc, tt * 512:(tt + 1) * 512],
                            start=(dc == 0), stop=(dc == 1))
                    nc.vector.tensor_scalar_add(
                        out=qT[:, fc, tt * 512:(tt + 1) * 512],
                        in0=ps, scalar1=bq_col[:, fc:fc + 1])
            # V' natural: [t(128), VAUG_COLS] per key chunk
            for kc in range(S // 128):
                ps = ppsum.tile([128, VAUG_COLS], F32, tag="projv")
                for dc in range(2):
                    nc.tensor.matmul(
                        ps,
                        lhsT=xT[:, dc, kc * 128:(kc + 1) * 128],
                        rhs=wva_sb[:, dc, :],
                        start=(dc == 0), stop=(dc == 1))
                nc.vector.tensor_add(out=v_sb[:, kc, :], in0=ps, in1=bva_bc)

        # ---- phase D: attention ---------------------------------------
        NKC = S // 128   # 32 key chunks
        NQB = SQ // 512  # 4 query blocks
        with tc.tile_pool(name="spsum", bufs=2, space="PSUM") as spsum, \
             tc.tile_pool(name="vpsum", bufs=1, space="PSUM") as vpsum, \
             tc.tile_pool(name="epsum", bufs=1, space="PSUM") as epsum, \
             tc.tile_pool(name="ptile", bufs=4) as ptile, \
             tc.tile_pool(name="etile", bufs=2) as etile, \
             tc.tile_pool(name="otile", bufs=2) as otile:
            for j in range(NQB):
                anorm = []
                for pair in range(2):
                    pv0 = vpsum.tile([65, 512], F32, tag="pv0")
                    pv1 = vpsum.tile([128, 512], F32, tag="pv1")
                    for kc in range(NKC):
                        # scores^T for the head pair, both in one 2-bank tile
                        ps = spsum.tile([128, 1024], F32, tag="ps")
                        nc.tensor.matmul(
                            ps[:, 0:512],
                            lhsT=kT[0:64, pair, kc * 128:(kc + 1) * 128],
                            rhs=qT[0:64, pair, j * 512:(j + 1) * 512],
                            start=True, stop=True)
                        nc.tensor.matmul(
                            ps[:, 512:1024],
                            lhsT=kT[64:128, pair, kc * 128:(kc + 1) * 128],
                            rhs=qT[64:128, pair, j * 512:(j + 1) * 512],
                            start=True, stop=True)
                        pt = ptile.tile([128, 1024], F32, tag="pt")
                        if kc in DVE_EXP_KCS:
                            op1, op2 = _register_exp_ops()
                            mid = ptile.tile([128, 1024], F32, tag="ptmid")
                            nc.vector._custom_dve(
                                op1, out=mid, in0=ps,
                                s0=1.0 / 1024.0, s1=1.0 / 6.0, imm2=0.5)
                            nc.vector._custom_dve(op2, out=pt, in0=mid)
                        else:
                            nc.scalar.activation(
                                out=pt, in_=ps,
                                func=mybir.ActivationFunctionType.Exp,
                                scale=0.125)
                        nc.tensor.matmul(
                            pv0,
                            lhsT=v_sb[:, kc, _even_off(pair):_even_off(pair) + 65],
                            rhs=pt[:, 0:512],
                            start=(kc == 0), stop=(kc == NKC - 1))
                        nc.tensor.matmul(
                            pv1,
                            lhsT=v_sb[:, kc, _odd_off(pair):_odd_off(pair) + 128],
                            rhs=pt[:, 512:1024],
                            start=(kc == 0), stop=(kc == NKC - 1))
                    # epilogue: stack attn pair, divide by l
                    astack = etile.tile([128, 512], F32, tag="astack")
                    nc.vector.tensor_copy(out=astack[0:64, :], in_=pv0[0:64, :])
                    nc.vector.tensor_copy(out=astack[64:128, :], in_=pv1[64:128, :])
                    # l_even sits at psum partition 64, l_odd at partition 32;
                    # copy to SBUF, replicate raw l across each head's 64 rows
                    # with K=1 ones-stationary matmuls, then 1/l = exp(-ln(l))
                    # on ACT (the Reciprocal ACT function is blocked, and the
                    # natural_log set holds exp too so no table switching)
                    lsb0 = etile.tile([65, 512], F32, tag="lsb0")
                    lsb1 = etile.tile([33, 512], F32, tag="lsb1")
                    nc.vector.tensor_copy(out=lsb0[64:65, :], in_=pv0[64:65, :])
                    nc.vector.tensor_copy(out=lsb1[32:33, :], in_=pv1[32:33, :])
                    pl = epsum.tile([128, 512], F32, tag="pl")
                    nc.tensor.matmul(
                        pl[0:64, :], lhsT=ones128[64:65, 0:64],
                        rhs=lsb0[64:65, :], start=True, stop=True,
                        tile_position=(64, 0))
                    nc.tensor.matmul(
                        pl[64:128, :], lhsT=ones128[32:33, 0:64],
                        rhs=lsb1[32:33, :], start=True, stop=True,
                        tile_position=(32, 64))
                    lnl = etile.tile([128, 512], F32, tag="lnl")
                    nc.scalar.activation(
                        out=lnl, in_=pl,
                        func=mybir.ActivationFunctionType.Ln)
                    linvrep = etile.tile([128, 512], F32, tag="linvrep")
                    nc.scalar.activation(
                        out=linvrep, in_=lnl,
                        func=mybir.ActivationFunctionType.Exp, scale=-1.0)
                    an = etile.tile([128, 512], F32, tag=f"anorm{pair}")
                    nc.vector.tensor_mul(out=an, in0=astack, in1=linvrep)
                    anorm.append(an)
                # output projection for this query block
                for tsub in range(4):
                    po = epsum.tile([128, D], F32, tag="po")
                    for pair in range(2):
                        nc.tensor.matmul(
                            po,
                            lhsT=anorm[pair][:, tsub * 128:(tsub + 1) * 128],
                            rhs=wo_sb[:, pair, :],
                            start=(pair == 0), stop=(pair == 1))
                    ot = otile.tile([128, D], F32, tag="ot")
                    nc.vector.tensor_add(out=ot, in0=po, in1=bo_bc)
                    t0 = j * 512 + tsub * 128
                    nc.sync.dma_start(out=out[t0:t0 + 128, :], in_=ot)


_NC_CACHE = None


def _get_program():
    global _NC_CACHE
    if _NC_CACHE is None:
        _NC_CACHE = build_program()
    return _NC_CACHE


def make_in_maps(inputs):
    """Build the 8 per-core input dicts from the full-problem inputs."""
    xs = np.ascontiguousarray(np.asarray(inputs["inputs"], np.float32))
    common = {
        "wq": np.ascontiguousarray(np.asarray(inputs["Wq"], np.float32)),
        "bq": np.ascontiguousarray(np.asarray(inputs["bq"], np.float32)),
        "wk": np.ascontiguousarray(np.asarray(inputs["Wk"], np.float32)),
        "bk": np.ascontiguousarray(np.asarray(inputs["bk"], np.float32)),
        "wv": np.ascontiguousarray(np.asarray(inputs["Wv"], np.float32)),
        "bv": np.ascontiguousarray(np.asarray(inputs["bv"], np.float32)),
        "wo": np.ascontiguousarray(np.asarray(inputs["Wo"], np.float32)),
        "bo": np.ascontiguousarray(np.asarray(inputs["bo"], np.float32)),
    }
    in_maps = []
    for c in range(NCORES):
        b, half = c // 2, c % 2
        xb = xs[b]
        x_rot = np.ascontiguousarray(np.roll(xb, -half * SQ, axis=0))
        in_maps.append({"x": x_rot, **common})
    return in_maps


def assemble_out(results):
    out = np.empty((B, S, D), np.float32)
    for c in range(NCORES):
        b, half = c // 2, c % 2
        out[b, half * SQ:(half + 1) * SQ, :] = results[c]["out"]
    return out


def kernel(**inputs):
    from concourse.bass_utils import run_bass_kernel_spmd

    nc = _get_program()
    in_maps = make_in_maps(inputs)
    res = run_bass_kernel_spmd(nc, in_maps, core_ids=list(range(NCORES)))
    return assemble_out(res.results)
